# revision 2
# baseline (speedup 1.0000x reference)
"""Trainium2 Bass kernel for nn_CrossAttention_249108103802.

8 cores data-parallel over B=8; per core (batch b):
  G_s   = x_s^T x_s           (Gram, fp32r full-rate MMs, upper triangle)
  A_s   = (G_s - mu I) Wv_s   (fp32r, free=512)
  ctp_s = A_s^T Wk_s + mu Wv_s^T Wk_s   (bf16 pair-packed + exact-fp32 TT)
  ctx_s = softmax_d(scale * ctp_s)      (per-head 64x64, DMA-XBAR transposed)
  o2    = x2 @ blockdiag(ctx1)  fused into the x2 streaming phase
  o1    = x1 @ blockdiag(ctx2)  trailing, from resident xT1

x is loaded fp32 via HWDGE (sync queue, fast 2KB packets); the only bf16
casts are on PSUM->SBUF copies that are needed anyway.
"""
import sys

sys.path.insert(0, "/opt/trn_rl_repo")

import numpy as np

import concourse.bass as bass
import concourse.mybir as mybir
import concourse.tile as tile
from concourse import bacc
from concourse.bass_utils import run_bass_kernel_spmd
from concourse.masks import make_identity

B, N, C, H = 8, 4096, 512, 8
HD = C // H                    # 64
SCALE = HD ** -0.5             # 1/8
MU = float(N)
NT = N // 128                  # 32 row tiles
CB = C // 128                  # 4 feature blocks
HP = H // 2                    # 4 head pairs
BF = mybir.dt.bfloat16
F32 = mybir.dt.float32
F32R = mybir.dt.float32r
AF = mybir.ActivationFunctionType

# Gram psum column ranges per row-block m (free >= 256 keeps fp32r at
# full rate; m=3 recomputes the (3,2) tile, which also saves a transpose)
GCOL = [(0, 512), (128, 512), (256, 512), (256, 512)]
# lower-triangle tiles still needing a PE transpose
LOWT = [(1, 0), (2, 0), (2, 1), (3, 0), (3, 1)]


def build():
    nc = bacc.Bacc("TRN2", target_bir_lowering=False, debug=False, num_devices=8)
    x_d = [nc.declare_dram_parameter("x1", [N, C], F32R, isOutput=False),
           nc.declare_dram_parameter("x2", [N, C], F32R, isOutput=False)]
    w_d = [nc.declare_dram_parameter("W_kv1", [C, 2 * C], F32R, isOutput=False),
           nc.declare_dram_parameter("W_kv2", [C, 2 * C], F32R, isOutput=False)]
    o_d = [nc.declare_dram_parameter("o1", [N, C], BF, isOutput=True),
           nc.declare_dram_parameter("o2", [N, C], BF, isOutput=True)]

    with tile.TileContext(nc) as tc:
        with (
            tc.tile_pool(name="const", bufs=1) as constp,
            tc.tile_pool(name="wf", bufs=1) as wfp,
            tc.tile_pool(name="whk", bufs=1) as whkp,
            tc.tile_pool(name="tts", bufs=1) as ttsp,
            tc.tile_pool(name="x", bufs=4) as xp,
            tc.tile_pool(name="xt", bufs=1) as xtp,
            tc.tile_pool(name="xts", bufs=2) as xtsp,
            tc.tile_pool(name="g", bufs=1) as gp_,
            tc.tile_pool(name="a", bufs=1) as ap_,
            tc.tile_pool(name="cx", bufs=1) as cxp,
            tc.tile_pool(name="osb", bufs=3) as osp,
            tc.tile_pool(name="ps_g", bufs=1, space="PSUM") as psg,
            tc.tile_pool(name="ps_t", bufs=2, space="PSUM") as pst,
            tc.tile_pool(name="ps_o", bufs=2, space="PSUM") as pso,
        ):
            identf = constp.tile([128, 128], F32, tag="identf")
            make_identity(nc, identf[:])
            ident = constp.tile([128, 128], F32R, tag="ident")
            nc.scalar.copy(ident[:], identf[:])
            zeros = constp.tile([128, HP, 128], F32, tag="zeros")
            nc.gpsimd.memset(zeros[:], 0.0)
            muI = constp.tile([128, 128], F32, tag="muI")
            nc.gpsimd.memset(muI[:], 0.0)
            nc.gpsimd.affine_select(
                out=muI[:], in_=muI[:],
                compare_op=mybir.AluOpType.not_equal, fill=MU,
                base=0, pattern=[[-1, 128]], channel_multiplier=1,
            )

            # ---- weight loads (scalar HWDGE queue) + bf16 Wk copies ----
            wfs, whks, ttss = [], [], []
            for s in range(2):
                wf = wfp.tile([128, CB, 2 * C], F32R, tag=f"wf{s}")
                nc.scalar.dma_start(
                    out=wf[:], in_=w_d[s][:, :].rearrange("(a p) m -> p a m", p=128))
                wfs.append(wf)
                whk = whkp.tile([128, CB, C], BF, tag=f"whk{s}")
                for a in range(CB):
                    nc.vector.tensor_copy(whk[:, a, :], wf[:, a, 0:C].bitcast(F32))
                whks.append(whk)
                ttss.append(ttsp.tile([128, HP, 128], F32, tag=f"tts{s}",
                                      name=f"tts{s}"))

            # ---- x loads: both tensors up front on the sync HWDGE queue;
            # ring of 4 1-MiB groups provides the prefetch window.
            xcs = {}
            for s in range(2):
                for r in range(NT // 4):
                    xc = xp.tile([128, 4, C], F32R, tag="xc", name=f"xc{s}_{r}")
                    nc.sync.dma_start(
                        out=xc[:],
                        in_=x_d[s][512 * r:512 * (r + 1), :].rearrange(
                            "(t p) c -> p t c", p=128))
                    xcs[(s, r)] = xc

            def tt_weights(s):
                # exact-fp32 TT = mu * Wv^T Wk, pair-packed [e(2h), d(2h)]
                wf = wfs[s]
                for hp in range(HP):
                    ttp = pso.tile([128, C], F32, tag="op", name=f"ttp{s}_{hp}")
                    for a in range(CB):
                        nc.tensor.matmul(
                            ttp[:, 0:128],
                            lhsT=wf[:, a, C + 128 * hp:C + 128 * (hp + 1)].bitcast(F32),
                            rhs=wf[:, a, 128 * hp:128 * (hp + 1)].bitcast(F32),
                            start=(a == 0), stop=(a == CB - 1))
                    nc.scalar.mul(ttss[s][:, hp, :], ttp[:, 0:128], MU)

            def xpose_copy(s, t, dst):
                """PE-transpose the 4 column blocks of x tile t into dst (bf16)."""
                xc = xcs[(s, t // 4)]
                tt_ = t % 4
                tp4 = pst.tile([128, CB, 128], F32R, tag="tp4", name=f"tp4_{s}_{t}")
                for cb in range(CB):
                    nc.tensor.transpose(
                        tp4[:, cb, :], xc[:, tt_, 128 * cb:128 * (cb + 1)],
                        ident[:])
                if t % 2 == 0:
                    nc.scalar.copy(dst, tp4[:].bitcast(F32))
                else:
                    nc.vector.tensor_copy(dst, tp4[:].bitcast(F32))

            def gram_mm(s, t, gps):
                xc = xcs[(s, t // 4)]
                tt_ = t % 4
                for m in range(CB):
                    lo, hi = GCOL[m]
                    nc.tensor.matmul(
                        gps[m][:],
                        lhsT=xc[:, tt_, 128 * m:128 * (m + 1)],
                        rhs=xc[:, tt_, lo:hi],
                        start=(t == 0), stop=(t == NT - 1))

            def ctx_tail(s, gps):
                """G psum -> Gc sbuf, A, ctp, softmax, cbd (bf16)."""
                wf = wfs[s]
                # Gc = G - mu I (fp32 sbuf), alternate engines
                gsb = gp_.tile([128, CB, C], F32R, tag="gsb", name=f"gsb{s}")
                eng = [nc.vector.tensor_copy, nc.scalar.copy]
                for m in range(CB):
                    lo, hi = GCOL[m]
                    dg = 128 * m - lo   # diag block offset inside psum tile
                    nc.vector.tensor_sub(
                        gsb[:, m, 128 * m:128 * (m + 1)],
                        gps[m][:, dg:dg + 128], muI[:])
                    if m < 3:
                        eng[m % 2](gsb[:, m, 128 * (m + 1):C],
                                   gps[m][:, dg + 128:hi - lo])
                    else:
                        # (3,2) tile came free from the widened m=3 psum
                        nc.scalar.copy(gsb[:, 3, 256:384], gps[3][:, 0:128])
                # lower-triangle tiles by PE transpose (plain fp32)
                gtr = gp_.tile([128, len(LOWT), 128], F32R, tag="gtr",
                               name=f"gtr{s}")
                for i, (a2, b2) in enumerate(LOWT):
                    tpg = pst.tile([128, CB, 128], F32R, tag="tp4",
                                   name=f"tpg{s}_{i}")
                    nc.tensor.transpose(
                        tpg[:, 0, :],
                        gsb[:, b2, 128 * a2:128 * (a2 + 1)], ident[:])
                    nc.vector.tensor_copy(gtr[:, i, :], tpg[:, 0, :])
                low = {ab: i for i, ab in enumerate(LOWT)}

                def g_tile(a2, b2):
                    if b2 >= a2:
                        return gsb[:, a2, 128 * b2:128 * (b2 + 1)]
                    if (a2, b2) == (3, 2):
                        return gsb[:, 3, 256:384]
                    return gtr[:, low[(a2, b2)], :]

                # A = Gc^T-tiles @ Wv (fp32r, free 512); cast to bf16
                ab = ap_.tile([128, CB, C], BF, tag="ab", name=f"ab{s}")
                for b2 in range(CB):
                    apx = pso.tile([128, C], F32, tag="op", name=f"apx{s}_{b2}")
                    for a2 in range(CB):
                        nc.tensor.matmul(
                            apx[:], lhsT=g_tile(a2, b2), rhs=wf[:, a2, C:2 * C],
                            start=(a2 == 0), stop=(a2 == CB - 1))
                    eng[b2 % 2](ab[:, b2, :], apx[:])

                # ctp (pair-packed) + TT, exp over valid halves, normalize
                esb = cxp.tile([128, HP, 128], F32, tag="esb", name=f"esb{s}")
                ssum = cxp.tile([128, HP], F32, tag="ssum", name=f"ssum{s}")
                rsum = cxp.tile([128, HP], F32, tag="rsum", name=f"rsum{s}")
                ctxts = cxp.tile([128, HP, 128], F32R, tag="ctxts",
                                 name=f"ctxts{s}")
                comb = cxp.tile([128, HP, 128], F32, tag="comb", name=f"comb{s}")
                for hp in range(HP):
                    ctp = pso.tile([128, C], F32, tag="op", name=f"ctp{s}_{hp}")
                    sl = slice(128 * hp, 128 * (hp + 1))
                    for b2 in range(CB):
                        nc.tensor.matmul(
                            ctp[:, 0:128], lhsT=ab[:, b2, sl], rhs=whks[s][:, b2, sl],
                            start=(b2 == 0), stop=(b2 == CB - 1))
                    nc.vector.tensor_add(comb[:, hp, :], ctp[:, 0:128],
                                         ttss[s][:, hp, :])
                    nc.scalar.activation(
                        esb[0:64, hp, 0:64], comb[0:64, hp, 0:64], AF.Exp,
                        scale=SCALE, accum_out=ssum[0:64, hp:hp + 1])
                    nc.scalar.activation(
                        esb[64:128, hp, 64:128], comb[64:128, hp, 64:128], AF.Exp,
                        scale=SCALE, accum_out=ssum[64:128, hp:hp + 1])
                nc.vector.reciprocal(rsum[:], ssum[:])
                cbd = cxp.tile([128, HP, 128], BF, tag=f"cbd{s}")
                # keep cross-head quadrants zero; only write diag halves so
                # the DMA-XBAR transpose below carries the zeros along
                nc.vector.tensor_copy(ctxts[:], zeros[:])
                for hp in range(HP):
                    nc.vector.tensor_scalar_mul(
                        ctxts[0:64, hp, 0:64], esb[0:64, hp, 0:64],
                        rsum[0:64, hp:hp + 1])
                    nc.vector.tensor_scalar_mul(
                        ctxts[64:128, hp, 64:128], esb[64:128, hp, 64:128],
                        rsum[64:128, hp:hp + 1])
                    # PE transpose [e,d] -> [d,e]; zero quadrants carry over
                    tpc = pst.tile([128, CB, 128], F32R, tag="tp4",
                                   name=f"tpc{s}_{hp}")
                    nc.tensor.transpose(tpc[:, 0, :], ctxts[:, hp, :], ident[:])
                    nc.scalar.copy(cbd[:, hp, :], tpc[:, 0, :].bitcast(F32))
                return cbd

            def out_mm(s, t, xt_sl, cbd, ob):
                """o_s tile t = xT tile @ blockdiag(ctx_other)."""
                op = pso.tile([128, C], F32, tag="op", name=f"op{s}_{t}")
                for cb in range(CB):
                    nc.tensor.matmul(
                        op[:, 128 * cb:128 * (cb + 1)],
                        lhsT=xt_sl(cb), rhs=cbd[:, cb, :], start=True, stop=True)
                if t % 2 == 0:
                    nc.vector.tensor_copy(ob[:, t % 4, :], op[:])
                else:
                    nc.scalar.copy(ob[:, t % 4, :], op[:])

            # ================= phase 1: x1 =================
            gps1 = [psg.tile([128, hi - lo], F32, tag=f"gp{m}", name=f"gp{m}_0")
                    for m, (lo, hi) in enumerate(GCOL)]
            xt1 = xtp.tile([128, CB, N], BF, tag="xt1")
            for t in range(NT):
                gram_mm(0, t, gps1)
                xpose_copy(0, t, xt1[:, :, 128 * t:128 * (t + 1)])
                if t == 3:
                    tt_weights(0)
                if t == 7:
                    tt_weights(1)
            cbd1 = ctx_tail(0, gps1)

            # ========== phase 2: x2 + fused o2 = x2 @ cbd1 ==========
            gps2 = [psg.tile([128, hi - lo], F32, tag=f"gp{m}", name=f"gp{m}_1")
                    for m, (lo, hi) in enumerate(GCOL)]
            xt2 = xtp.tile([128, CB, N], BF, tag="xt2")
            for r in range(NT // 4):
                ob = osp.tile([128, 4, C], BF, tag="ob", name=f"ob2_{r}")
                for tt_ in range(4):
                    t = 4 * r + tt_
                    gram_mm(1, t, gps2)
                    xpose_copy(1, t, xt2[:, :, 128 * t:128 * (t + 1)])
                    out_mm(1, t,
                           lambda cb: xt2[:, cb, 128 * t:128 * (t + 1)], cbd1, ob)
                nc.scalar.dma_start(
                    out=o_d[1][512 * r:512 * (r + 1), :].rearrange(
                        "(t p) c -> p t c", p=128),
                    in_=ob[:])
            cbd2 = ctx_tail(1, gps2)

            # ============ phase 3: o1 = x1 @ cbd2 (resident xT1) ============
            for r in range(NT // 4):
                ob = osp.tile([128, 4, C], BF, tag="ob", name=f"ob1_{r}")
                for tt_ in range(4):
                    t = 4 * r + tt_
                    out_mm(0, t,
                           lambda cb: xt1[:, cb, 128 * t:128 * (t + 1)], cbd2, ob)
                nc.sync.dma_start(
                    out=o_d[0][512 * r:512 * (r + 1), :].rearrange(
                        "(t p) c -> p t c", p=128),
                    in_=ob[:])
    nc.compile()
    return nc


_NC = None


def make_in_maps(inputs):
    x1 = np.ascontiguousarray(inputs["x1"], dtype=np.float32)
    x2 = np.ascontiguousarray(inputs["x2"], dtype=np.float32)
    W1 = np.ascontiguousarray(inputs["W_kv1"], dtype=np.float32)
    W2 = np.ascontiguousarray(inputs["W_kv2"], dtype=np.float32)
    return [
        {"x1": x1[b], "x2": x2[b], "W_kv1": W1, "W_kv2": W2} for b in range(B)
    ]


def kernel(x1, x2, W_kv1, W_kv2):
    global _NC
    if _NC is None:
        _NC = build()
    in_maps = make_in_maps(
        {"x1": x1, "x2": x2, "W_kv1": W_kv1, "W_kv2": W_kv2})
    res = run_bass_kernel_spmd(_NC, in_maps, core_ids=list(range(B)))
    o1 = np.stack([res.results[b]["o1"].astype(np.float32) for b in range(B)])
    o2 = np.stack([res.results[b]["o2"].astype(np.float32) for b in range(B)])
    return o1, o2



# revision 3
# speedup vs baseline: 1.2136x; 1.2136x over previous
"""Trainium2 Bass kernel for nn_CrossAttention_249108103802.

8 cores data-parallel over B=8; per core (batch b):
  G_s   = x_s^T x_s            (Gram, fp16 operands, fp32 psum, upper tri)
  A_s   = (G_s - mu I) Wv_s    (fp16)
  ctp_s = A_s^T Wk_s + mu Wv_s^T Wk_s   (fp16 pair-packed + fp32 TT)
  ctx_s = softmax_d(scale * ctp_s)      (per-head 64x64)
  o2^T  = blockdiag(ctx1)^T-stationary @ xT2   fused into x2 streaming
  o1^T  = same with ctx2 / resident xT1

Host supplies x, xT (pre-transposed) and W all in fp16; outputs are
written as o^T [C, N] fp16 and transposed back on the host. This
removes every PE transpose of x and keeps the out-projection matmuls
512 columns wide with a stationary 128x128 ctx block.
"""
import sys

sys.path.insert(0, "/opt/trn_rl_repo")

import numpy as np

import concourse.bass as bass
import concourse.mybir as mybir
import concourse.tile as tile
from concourse import bacc
from concourse.bass_utils import run_bass_kernel_spmd
from concourse.masks import make_identity

B, N, C, H = 8, 4096, 512, 8
HD = C // H                    # 64
SCALE = HD ** -0.5             # 1/8
MU = float(N)
NT = N // 128                  # 32 row tiles
CB = C // 128                  # 4 feature blocks
HP = H // 2                    # 4 head pairs
NG = 4                         # 1024-row / 1024-col streaming groups
GW = N // NG                   # 1024
TPG = NT // NG                 # 8 row tiles per group
F16 = mybir.dt.float16
F32 = mybir.dt.float32
AF = mybir.ActivationFunctionType

# Gram psum column ranges per row-block m (upper triangle; m=3 widened to
# recompute the (3,2) tile, which saves a transpose)
GCOL = [(0, 512), (128, 512), (256, 512), (256, 512)]
# lower-triangle tiles still needing a PE transpose
LOWT = [(1, 0), (2, 0), (2, 1), (3, 0), (3, 1)]


def build():
    nc = bacc.Bacc("TRN2", target_bir_lowering=False, debug=False, num_devices=8)
    x_d = [nc.declare_dram_parameter(f"x{s + 1}", [N, C], F16, isOutput=False)
           for s in range(2)]
    xt_d = [nc.declare_dram_parameter(f"xt{s + 1}", [C, N], F16, isOutput=False)
            for s in range(2)]
    w_d = [nc.declare_dram_parameter(f"w{s + 1}", [C, 2 * C], F16, isOutput=False)
           for s in range(2)]
    o_d = [nc.declare_dram_parameter(f"ot{s + 1}", [C, N], F16, isOutput=True)
           for s in range(2)]

    with tile.TileContext(nc) as tc:
        with (
            tc.tile_pool(name="const", bufs=1) as constp,
            tc.tile_pool(name="wf", bufs=1) as wfp,
            tc.tile_pool(name="tts", bufs=1) as ttsp,
            tc.tile_pool(name="x", bufs=4) as xp,
            tc.tile_pool(name="xt2", bufs=2) as xt2p,
            tc.tile_pool(name="xt1", bufs=1) as xt1p,
            tc.tile_pool(name="g", bufs=1) as gp_,
            tc.tile_pool(name="a", bufs=1) as ap_,
            tc.tile_pool(name="cx", bufs=1) as cxp,
            tc.tile_pool(name="osb", bufs=2) as osp,
            tc.tile_pool(name="ps_g", bufs=1, space="PSUM") as psg,
            tc.tile_pool(name="ps_t", bufs=2, space="PSUM") as pst,
            tc.tile_pool(name="ps_o", bufs=2, space="PSUM") as pso,
        ):
            identf = constp.tile([128, 128], F32, tag="identf")
            make_identity(nc, identf[:])
            ident16 = constp.tile([128, 128], F16, tag="ident16")
            nc.scalar.copy(ident16[:], identf[:])
            muI = constp.tile([128, 128], F32, tag="muI")
            nc.gpsimd.memset(muI[:], 0.0)
            nc.gpsimd.affine_select(
                out=muI[:], in_=muI[:],
                compare_op=mybir.AluOpType.not_equal, fill=MU,
                base=0, pattern=[[-1, 128]], channel_multiplier=1,
            )

            # ---- weight + resident-xT1 loads (scalar HWDGE queue) ----
            wfs, ttss = [], []
            for s in range(2):
                wf = wfp.tile([128, CB, 2 * C], F16, tag=f"wf{s}")
                nc.scalar.dma_start(
                    out=wf[:], in_=w_d[s][:, :].rearrange("(a p) m -> p a m", p=128))
                wfs.append(wf)
                ttss.append(ttsp.tile([128, HP, 128], F32, tag=f"tts{s}",
                                      name=f"tts{s}"))
            xt1 = xt1p.tile([128, CB, N], F16, tag="xt1")
            for g in range(NG):
                nc.scalar.dma_start(
                    out=xt1[:, :, GW * g:GW * (g + 1)],
                    in_=xt_d[0][:, GW * g:GW * (g + 1)].rearrange(
                        "(a p) n -> p a n", p=128))

            # ---- x / xT2 streaming loads (sync HWDGE queue) ----
            xcs = {}
            xt2s = {}
            for g in range(NG):
                xc = xp.tile([128, TPG, C], F16, tag="xc", name=f"xc0_{g}")
                nc.sync.dma_start(
                    out=xc[:],
                    in_=x_d[0][GW * g:GW * (g + 1), :].rearrange(
                        "(t p) c -> p t c", p=128))
                xcs[(0, g)] = xc
            for g in range(NG):
                xc = xp.tile([128, TPG, C], F16, tag="xc", name=f"xc1_{g}")
                nc.sync.dma_start(
                    out=xc[:],
                    in_=x_d[1][GW * g:GW * (g + 1), :].rearrange(
                        "(t p) c -> p t c", p=128))
                xcs[(1, g)] = xc
                xt2c = xt2p.tile([128, CB, GW], F16, tag="xt2", name=f"xt2_{g}")
                nc.sync.dma_start(
                    out=xt2c[:],
                    in_=xt_d[1][:, GW * g:GW * (g + 1)].rearrange(
                        "(a p) n -> p a n", p=128))
                xt2s[g] = xt2c

            eng = [nc.vector.tensor_copy, nc.scalar.copy]

            def tt_weights(s):
                # exact TT = mu * Wv^T Wk, pair-packed [e(2h), d(2h)]
                wf = wfs[s]
                for hp in range(HP):
                    ttp = pso.tile([128, C], F32, tag="op", name=f"ttp{s}_{hp}")
                    for a in range(CB):
                        nc.tensor.matmul(
                            ttp[:, 0:128],
                            lhsT=wf[:, a, C + 128 * hp:C + 128 * (hp + 1)],
                            rhs=wf[:, a, 128 * hp:128 * (hp + 1)],
                            start=(a == 0), stop=(a == CB - 1))
                    nc.scalar.mul(ttss[s][:, hp, :], ttp[:, 0:128], MU)

            def gram_mm(s, t, gps):
                xc = xcs[(s, t // TPG)]
                tt_ = t % TPG
                for m in range(CB):
                    lo, hi = GCOL[m]
                    nc.tensor.matmul(
                        gps[m][:],
                        lhsT=xc[:, tt_, 128 * m:128 * (m + 1)],
                        rhs=xc[:, tt_, lo:hi],
                        start=(t == 0), stop=(t == NT - 1))

            def ctx_tail(s, gps):
                """G psum -> Gc sbuf (f16), A, ctp, softmax, cbd (f16)."""
                wf = wfs[s]
                gsb = gp_.tile([128, CB, C], F16, tag="gsb", name=f"gsb{s}")
                for m in range(CB):
                    lo, hi = GCOL[m]
                    dg = 128 * m - lo   # diag block offset inside psum tile
                    nc.vector.tensor_sub(
                        gsb[:, m, 128 * m:128 * (m + 1)],
                        gps[m][:, dg:dg + 128], muI[:])
                    if m < 3:
                        eng[m % 2](gsb[:, m, 128 * (m + 1):C],
                                   gps[m][:, dg + 128:hi - lo])
                    else:
                        # (3,2) tile came free from the widened m=3 psum
                        nc.scalar.copy(gsb[:, 3, 256:384], gps[3][:, 0:128])
                # lower-triangle tiles by PE transpose (f16)
                gtr = gp_.tile([128, len(LOWT), 128], F16, tag="gtr",
                               name=f"gtr{s}")
                for i, (a2, b2) in enumerate(LOWT):
                    tpg = pst.tile([128, 128], F16, tag="tp", name=f"tpg{s}_{i}")
                    nc.tensor.transpose(
                        tpg[:], gsb[:, b2, 128 * a2:128 * (a2 + 1)], ident16[:])
                    nc.vector.tensor_copy(gtr[:, i, :], tpg[:])
                low = {ab: i for i, ab in enumerate(LOWT)}

                def g_tile(a2, b2):
                    if b2 >= a2:
                        return gsb[:, a2, 128 * b2:128 * (b2 + 1)]
                    if (a2, b2) == (3, 2):
                        return gsb[:, 3, 256:384]
                    return gtr[:, low[(a2, b2)], :]

                # A = Gc^T-tiles @ Wv (f16, free 512)
                ab = ap_.tile([128, CB, C], F16, tag="ab", name=f"ab{s}")
                for b2 in range(CB):
                    apx = pso.tile([128, C], F32, tag="op", name=f"apx{s}_{b2}")
                    for a2 in range(CB):
                        nc.tensor.matmul(
                            apx[:], lhsT=g_tile(a2, b2), rhs=wf[:, a2, C:2 * C],
                            start=(a2 == 0), stop=(a2 == CB - 1))
                    eng[b2 % 2](ab[:, b2, :], apx[:])

                # ctp (pair-packed) + TT, exp over valid halves, normalize
                esb = cxp.tile([128, HP, 128], F32, tag="esb", name=f"esb{s}")
                ssum = cxp.tile([128, HP], F32, tag="ssum", name=f"ssum{s}")
                rsum = cxp.tile([128, HP], F32, tag="rsum", name=f"rsum{s}")
                comb = cxp.tile([128, HP, 128], F32, tag="comb", name=f"comb{s}")
                ctxts = cxp.tile([128, HP, 128], F16, tag="ctxts",
                                 name=f"ctxts{s}")
                nc.gpsimd.memset(ctxts[:], 0.0)
                for hp in range(HP):
                    ctp = pso.tile([128, C], F32, tag="op", name=f"ctp{s}_{hp}")
                    sl = slice(128 * hp, 128 * (hp + 1))
                    for b2 in range(CB):
                        nc.tensor.matmul(
                            ctp[:, 0:128], lhsT=ab[:, b2, sl], rhs=wf[:, b2, sl],
                            start=(b2 == 0), stop=(b2 == CB - 1))
                    nc.vector.tensor_add(comb[:, hp, :], ctp[:, 0:128],
                                         ttss[s][:, hp, :])
                    nc.scalar.activation(
                        esb[0:64, hp, 0:64], comb[0:64, hp, 0:64], AF.Exp,
                        scale=SCALE, accum_out=ssum[0:64, hp:hp + 1])
                    nc.scalar.activation(
                        esb[64:128, hp, 64:128], comb[64:128, hp, 64:128], AF.Exp,
                        scale=SCALE, accum_out=ssum[64:128, hp:hp + 1])
                nc.vector.reciprocal(rsum[:], ssum[:])
                cbd = cxp.tile([128, HP, 128], F16, tag=f"cbd{s}")
                # only diag halves written; zero quadrants carry through the
                # PE transpose below
                for hp in range(HP):
                    nc.vector.tensor_scalar_mul(
                        ctxts[0:64, hp, 0:64], esb[0:64, hp, 0:64],
                        rsum[0:64, hp:hp + 1])
                    nc.vector.tensor_scalar_mul(
                        ctxts[64:128, hp, 64:128], esb[64:128, hp, 64:128],
                        rsum[64:128, hp:hp + 1])
                    tpc = pst.tile([128, 128], F16, tag="tp", name=f"tpc{s}_{hp}")
                    nc.tensor.transpose(tpc[:], ctxts[:, hp, :], ident16[:])
                    nc.scalar.copy(cbd[:, hp, :], tpc[:])
                return cbd

            def out_t(s, g, cbd, xt_sl):
                """o_s^T group g = blockdiag(ctx_other) stationary @ xT."""
                ob = osp.tile([128, CB, GW], F16, tag="ob", name=f"ob{s}_{g}")
                for cb in range(CB):
                    for h2 in range(GW // 512):
                        op = pso.tile([128, C], F32, tag="op",
                                      name=f"op{s}_{g}_{cb}_{h2}")
                        nc.tensor.matmul(
                            op[:], lhsT=cbd[:, cb, :], rhs=xt_sl(cb, g, h2),
                            start=True, stop=True)
                        eng[(cb + h2) % 2](ob[:, cb, 512 * h2:512 * (h2 + 1)],
                                           op[:])
                nc.scalar.dma_start(
                    out=o_d[s][:, GW * g:GW * (g + 1)].rearrange(
                        "(a p) n -> p a n", p=128),
                    in_=ob[:])

            # ================= phase 1: x1 gram =================
            gps1 = [psg.tile([128, hi - lo], F32, tag=f"gp{m}", name=f"gp{m}_0")
                    for m, (lo, hi) in enumerate(GCOL)]
            for t in range(NT):
                gram_mm(0, t, gps1)
                if t == 3:
                    tt_weights(0)
                if t == 7:
                    tt_weights(1)
            cbd1 = ctx_tail(0, gps1)

            # ========== phase 2: x2 gram + fused o2^T ==========
            gps2 = [psg.tile([128, hi - lo], F32, tag=f"gp{m}", name=f"gp{m}_1")
                    for m, (lo, hi) in enumerate(GCOL)]
            for g in range(NG):
                for tt_ in range(TPG):
                    gram_mm(1, TPG * g + tt_, gps2)
                out_t(1, g, cbd1,
                      lambda cb, g_, h2: xt2s[g_][:, cb, 512 * h2:512 * (h2 + 1)])
            cbd2 = ctx_tail(1, gps2)

            # ============ phase 3: o1^T from resident xT1 ============
            for g in range(NG):
                out_t(0, g, cbd2,
                      lambda cb, g_, h2: xt1[:, cb,
                                             GW * g_ + 512 * h2:
                                             GW * g_ + 512 * (h2 + 1)])
    nc.compile()
    return nc


_NC = None


def make_in_maps(inputs):
    x1 = np.asarray(inputs["x1"])
    x2 = np.asarray(inputs["x2"])
    w1 = np.ascontiguousarray(np.asarray(inputs["W_kv1"]), dtype=np.float16)
    w2 = np.ascontiguousarray(np.asarray(inputs["W_kv2"]), dtype=np.float16)
    in_maps = []
    for b in range(B):
        x1b = np.ascontiguousarray(x1[b], dtype=np.float16)
        x2b = np.ascontiguousarray(x2[b], dtype=np.float16)
        in_maps.append({
            "x1": x1b, "x2": x2b,
            "xt1": np.ascontiguousarray(x1b.T),
            "xt2": np.ascontiguousarray(x2b.T),
            "w1": w1, "w2": w2,
        })
    return in_maps


def kernel(x1, x2, W_kv1, W_kv2):
    global _NC
    if _NC is None:
        _NC = build()
    in_maps = make_in_maps(
        {"x1": x1, "x2": x2, "W_kv1": W_kv1, "W_kv2": W_kv2})
    res = run_bass_kernel_spmd(_NC, in_maps, core_ids=list(range(B)))
    o1 = np.stack([res.results[b]["ot1"].astype(np.float32).T
                   for b in range(B)])
    o2 = np.stack([res.results[b]["ot2"].astype(np.float32).T
                   for b in range(B)])
    return o1, o2
